# revision 3
# baseline (speedup 1.0000x reference)
"""Trainium2 Bass kernel for nn_GatedAtomUpdate (gnn_message_passing), v3.

Engine-balanced redesign. Per-core work is ~198k bond-slots (1.5M/8 plus
~4-5% tile padding); the four activation stages are the hard cost
(256 LUT-ish elems/bond), so the design minimizes ACT free-dim traffic
and moves the final gate sigmoid to a custom DVE op:

  - Host sorts bonds by receiver atom into 8 equal atom ranges
    (12500 atoms/core), buckets them into 128-atom blocks (98/core) and
    128-bond tiles (shared SPMD schedule = max count over cores).
  - Host also builds the one-hot scatter matrix S [128, ntiles*128] in
    bf16 and streams it by DMA (frees ~0.7ns/bond of DVE is_equal work).
  - L1 feature-major: psum1[128 h|g, 512] = [W1|G1]^T @ xt, ONE ACT silu
    per 512 bonds covering both halves.
  - L2 bond-major (stationary = act1 slice per tile, moving =
    blockdiag(W2,G2)): psum2[128 bonds, 8, 128 h2|g2]. ACT silu on the h
    half only (FD=512/1024 bonds).
  - Gate: custom DVE op SMOOTHSTEP_SIG_ANT computes
    s = clamp(a*x+0.5, 0, 1); sig = s^2*(3-2s)  (max |err vs sigmoid|
    = 0.0297, within the 2e-2 absmax budget after segment summing).
  - msg = h2 * sig on DVE (bf16 2x).
  - Segment-sum: per tile, PE matmul msg^T @ S accumulates [64 feat, 128
    atom] in PSUM; pseg double-buffers in the two partition halves of a
    single PSUM bank (blocks alternate parity). On block close, DVE adds
    the atom_features slice into out_sb.

PSUM budget: psum1 [128,512]x3 + psum2 [128,8,128]x2 + pseg [128,128]x1
= 3 + 4 + 1 = 8 banks exactly.
"""

import math

import numpy as np
import ml_dtypes

import bass_rust
import concourse.bass as bass
import concourse.mybir as mybir
import concourse.tile as tile

# ---------------------------------------------------------------- constants
N_CORES = 8
D = 64
N_ATOMS = 100000
N_BONDS = 1500000
NA_CORE = N_ATOMS // N_CORES          # 12500
BLK = 128                             # atoms per block / S width
NBLK = math.ceil(NA_CORE / BLK)       # 98 blocks/core
NA_PAD = NBLK * BLK                   # 12544
TPB = 128                             # bonds per tile
L2B = 8                               # tiles per batch (1024 bonds)
CHUNK_TILES = 64                      # tiles per DMA chunk (8 batches)

A_SS = 0.13790085                     # smoothstep-sigmoid slope (minimax fit)
ACT_GATE_EVERY = 0                    # 0 = all batches use the DVE gate
# clamped odd cubic-with-|x| sigmoid: clamp(0.5 + x(c1 + c2|x| + c3 x^2), 0, 1)
# minimax fit on [-20, 20]: max |err vs sigmoid| = 0.0053
C1A, C2A, C3A = 0.28057814, -0.05291577, 0.00330433
DVE_GATE_NUM = 11                     # hybrid: NUM-of-DEN batches on the DVE gate
DVE_GATE_DEN = 16

_BF16 = mybir.dt.bfloat16
_F32 = mybir.dt.float32
_FP8 = mybir.dt.float8e4


# ------------------------------------------------------- walrus workaround
def _patched_drain_and_barrier(self, tick_clock, wait_clock):
    """This walrus build accepts at most ONE sync-wait on TPB_CTRL-class
    instructions (Drain/NoOp); emit exit-drain waits on single-wait NOPs."""
    nc = self.nc
    gc = tick_clock.global_clock
    ticks = list(gc)
    n = len(ticks)
    for proc, t in enumerate(ticks):
        if t > 0:
            vcp = bass_rust.VectorClock([t if j == proc else 0 for j in range(n)])
            nop = nc.sync.nop()
            wait_clock.add_sem_waits(nop.ins, tile.ScopedClock({None: vcp}))
    nc.sync.drain()
    nc.all_engine_barrier()
    assert self.sems is not None
    popped = nc._tile_sem_poison_stack.pop()
    assert popped is self._sem_poison
    for s in list(self.sems.allocated().values()):
        nc.clear_and_free_semaphores([s])
    nc.all_engine_barrier()


tile.TileContext._drain_and_barrier = _patched_drain_and_barrier


def _split_multi_waits(bir):
    """Move extra sync-waits onto fresh single-wait NoOps (walrus rejects
    >1 wait per instruction)."""
    n_new = 0
    for fn in bir.get("functions", []):
        for bb in fn.get("blocks", []):
            insts = bb.get("instructions", [])
            out = []
            for inst in insts:
                si = inst.get("sync_info") or {}
                ow = si.get("on_wait") or []
                if len(ow) > 1:
                    for i, w in enumerate(ow[:-1]):
                        out.append({
                            "name": f"{inst['name']}_sw{i}",
                            "opcode": "NoOp",
                            "engine": inst["engine"],
                            "ins": [],
                            "outs": [],
                            "sync_info": {"on_update": [], "on_wait": [w]},
                            "debug": inst.get("debug", 0),
                        })
                        n_new += 1
                    si["on_wait"] = [ow[-1]]
                out.append(inst)
            bb["instructions"] = out
    return n_new


_orig_to_json_bytes = bass.Bass.to_json_bytes


def _to_json_bytes_patched(self, *args, **kwargs):
    import json as _json
    raw = _orig_to_json_bytes(self, *args, **kwargs)
    bir = _json.loads(raw)
    n = _split_multi_waits(bir)
    if n == 0:
        return raw
    return _json.dumps(bir).encode()


bass.Bass.to_json_bytes = _to_json_bytes_patched


# ------------------------------------------------- custom DVE gate sigmoid
_SS_OP = None


def _register_ss_op():
    """Register SMOOTHSTEP_SIG_ANT: out = s^2*(imm2*s + C3-latched-src1)
    with s = clamp(s0*in0 + s1, 0, 1). Called with s0=a, s1=0.5, imm2=-2,
    in1 = [P,1] tile holding 3.0."""
    global _SS_OP
    if _SS_OP is not None:
        return _SS_OP
    from concourse.dve_spec import (
        Spec, Src0, C0, C1, C2, C3, One, relu, sq, minn, lower,
        _spill_c3_to_src1,
    )
    from concourse.dve_ops import DveOp, OPS, CUSTOM_DVE_SPECS, _SUB_OPCODE_FOR_NAME
    from concourse.dve_table_gen import dve_ver_for
    from concourse.dve_uop import DveOpSpec

    name = "SMOOTHSTEP_SIG_ANT"
    if name in _SUB_OPCODE_FOR_NAME:
        _SS_OP = next(op for op in OPS if op.name == name)
        return _SS_OP

    s = minn(relu(Src0 * C0 + C1), One)
    body = _spill_c3_to_src1(sq(s) * (s * C2 + C3))

    def _ref(in0, in1, s0, s1, imm2):
        sc = np.minimum(np.maximum(in0 * s0 + s1, 0.0), 1.0)
        return sc * sc * (imm2 * sc + np.asarray(in1)[..., :1])

    spec = Spec(body=body, reference=_ref)
    ver = dve_ver_for("TRN2")
    row = 1 + len(OPS)
    assert row < 0x20, "custom-DVE opcode rows exhausted"
    sha = DveOpSpec(name=name, opcode=row, uops=lower(spec, ver=ver),
                    rd1_en=True).sha(ver)
    op = DveOp(name, spec, subdim=False, uops_sha={ver: sha})
    OPS.append(op)
    CUSTOM_DVE_SPECS[name] = spec
    _SUB_OPCODE_FOR_NAME[name] = row
    _SS_OP = op
    return op


# ------------------------------------------------------------ host sharding
NBLK_G = N_CORES * NBLK               # 784 global 128-atom blocks


def _plan_and_pack(atom_features, bond_features, bond_atom_indices, pad_to=48):
    """Assign 128-atom blocks to (core, position) slots by sorted-count
    round-robin (blocks of similar bond count share a position across cores,
    minimizing the shared-SPMD-schedule tile padding), then pack per-core
    xt (bf16 feature-major), the one-hot S (bf16), and position-ordered
    atom features. Returns `assign` [NBLK, N_CORES] -> global block id for
    output reassembly."""
    recv = bond_atom_indices[:, 1].astype(np.int64)
    blk_g = recv // BLK
    nb = np.bincount(blk_g, minlength=NBLK_G)

    order = np.argsort(-nb, kind="stable")
    assign = order.reshape(NBLK, N_CORES).copy()   # [position, core]
    assign[1::2] = assign[1::2, ::-1]              # snake for bond balance

    T = np.maximum(1, -(-nb[assign].max(axis=1) // TPB))
    pad_tiles = (-int(T.sum())) % pad_to
    T[-1] += pad_tiles
    ntiles = int(T.sum())
    tstart = np.concatenate([[0], np.cumsum(T)[:-1]]).astype(np.int64)
    tile_block = np.repeat(np.arange(NBLK), T)     # position ids per tile

    core_of_block = np.empty(NBLK_G, dtype=np.int64)
    pos_of_block = np.empty(NBLK_G, dtype=np.int64)
    core_of_block[assign.ravel()] = np.tile(np.arange(N_CORES), NBLK)
    pos_of_block[assign.ravel()] = np.repeat(np.arange(NBLK), N_CORES)

    bcore = core_of_block[blk_g]
    bpos = pos_of_block[blk_g]
    key = bcore * NBLK + bpos
    bond_order = np.argsort(key, kind="stable")
    sorted_key = key[bond_order]
    runs = np.bincount(key, minlength=N_CORES * NBLK)
    run_off = np.concatenate([[0], np.cumsum(runs)[:-1]])
    off_in_run = np.arange(recv.shape[0]) - run_off[sorted_key]
    pos_sorted = bpos[bond_order]
    slot = tstart[pos_sorted] * TPB + off_in_run
    rel_sorted = (recv - blk_g * BLK)[bond_order]
    core_sorted = bcore[bond_order]

    nslots = ntiles * TPB
    blk_atoms = np.arange(BLK)
    xt_list, S_list, atomT_list = [], [], []
    for c in range(N_CORES):
        m = core_sorted == c
        gidx = bond_order[m]
        sl = slot[m]
        rl = rel_sorted[m]

        x_slot = np.zeros((nslots, D), dtype=np.float32)
        x_slot[sl] = bond_features[gidx]
        xt_list.append(np.ascontiguousarray(x_slot.T).astype(ml_dtypes.bfloat16))

        S = np.zeros((TPB, ntiles * BLK), dtype=ml_dtypes.float8_e4m3)
        S[sl % TPB, (sl // TPB) * BLK + rl] = 1.0
        S_list.append(S)

        atom_idx = (assign[:, c][:, None] * BLK + blk_atoms[None, :]).ravel()
        valid = atom_idx < N_ATOMS
        ap = np.zeros((NA_PAD, D), dtype=np.float32)
        ap[valid] = atom_features[atom_idx[valid]]
        atomT_list.append(np.ascontiguousarray(ap.T))

    return ntiles, tile_block, xt_list, S_list, atomT_list, assign


def _pack_weights(W1, G1, W2, G2, b1, g1, b2, g2):
    wg1_row = np.concatenate([W1, G1], axis=1)              # [64, 128]
    wg1 = np.concatenate([wg1_row, wg1_row], axis=0)        # [128, 128]
    wg2 = np.zeros((2 * D, 2 * D), dtype=np.float32)
    wg2[:D, :D] = W2
    wg2[D:, D:] = G2
    b1g1 = np.concatenate([b1, g1]).reshape(2 * D, 1).astype(np.float32)
    b2g2 = np.concatenate([b2, g2]).reshape(1, 2 * D)
    return (
        wg1.astype(ml_dtypes.bfloat16),
        wg2.astype(ml_dtypes.bfloat16),
        b1g1,
        b2g2.astype(ml_dtypes.bfloat16),
    )


# ------------------------------------------------------------- device kernel
def _build_nc(ntiles, tile_block, has_bias2, gate_mode="hybrid",
              l2b=L2B, p1_cols=512, p1_bufs=3, p2_bufs=2, split_hg=False):
    nb2 = ntiles // l2b
    nc = bass.Bass()

    xt_d = nc.dram_tensor("xt", [D, ntiles * TPB], _BF16, kind="ExternalInput")
    S_d = nc.dram_tensor("smat", [TPB, ntiles * BLK], _FP8, kind="ExternalInput")
    atomT_d = nc.dram_tensor("atomT", [D, NA_PAD], _F32, kind="ExternalInput")
    wg1_d = nc.dram_tensor("wg1", [2 * D, 2 * D], _BF16, kind="ExternalInput")
    wg2_d = nc.dram_tensor("wg2", [2 * D, 2 * D], _BF16, kind="ExternalInput")
    b1g1_d = nc.dram_tensor("b1g1", [2 * D, 1], _F32, kind="ExternalInput")
    b2g2_d = nc.dram_tensor("b2g2", [1, 2 * D], _BF16, kind="ExternalInput")
    out_d = nc.dram_tensor("out", [D, NA_PAD], _F32, kind="ExternalOutput")

    AF = mybir.ActivationFunctionType

    first_of_block = np.zeros(ntiles, dtype=bool)
    last_of_block = np.zeros(ntiles, dtype=bool)
    prev = -1
    for t in range(ntiles):
        b = tile_block[t]
        if b != prev:
            first_of_block[t] = True
            if t > 0:
                last_of_block[t - 1] = True
            prev = b
    last_of_block[ntiles - 1] = True

    chunk_tiles = CHUNK_TILES
    while chunk_tiles % l2b != 0:
        chunk_tiles += l2b - (chunk_tiles % l2b)

    with tile.TileContext(nc) as tc:
        with (
            tc.tile_pool(name="singles", bufs=1) as singles,
            tc.tile_pool(name="xtp", bufs=2) as xtp,
            tc.tile_pool(name="smp", bufs=2) as smp,
            tc.tile_pool(name="actp", bufs=3) as actp,
            tc.tile_pool(name="h2p", bufs=2) as h2p,
            tc.tile_pool(name="sgp", bufs=2) as sgp,
            tc.tile_pool(name="msgp", bufs=2) as msgp,
            tc.tile_pool(name="psum1p", bufs=p1_bufs, space="PSUM") as psum1p,
            tc.tile_pool(name="psum2p", bufs=p2_bufs, space="PSUM") as psum2p,
            tc.tile_pool(name="psegp", bufs=1, space="PSUM") as psegp,
        ):
            wg1_sb = singles.tile([2 * D, 2 * D], _BF16)
            nc.sync.dma_start(out=wg1_sb[:], in_=wg1_d[:, :])
            wg2_sb = singles.tile([2 * D, 2 * D], _BF16)
            nc.sync.dma_start(out=wg2_sb[:], in_=wg2_d[:, :])
            b1g1_sb = singles.tile([2 * D, 1], _F32)
            nc.sync.dma_start(out=b1g1_sb[:], in_=b1g1_d[:, :])
            b2g2_sb = singles.tile([1, 2 * D], _BF16)
            nc.sync.dma_start(out=b2g2_sb[:], in_=b2g2_d[:, :])
            ones_sb = singles.tile([1, 2 * D], _BF16)
            nc.vector.memset(ones_sb[:], 1.0)
            c3_sb = singles.tile([2 * D, 1], _F32)
            nc.vector.memset(c3_sb[:], 3.0)
            atom_sb = singles.tile([D, NA_PAD], _F32)
            out_sb = singles.tile([D, NA_PAD], _F32)

            pseg2 = psegp.tile([TPB, BLK], _F32)

            stash = {}

            def emit_seg(j):
                msg_j, sm_sb_j, base_j = stash.pop(j)
                for tt in range(l2b):
                    t_glob = j * l2b + tt
                    b = int(tile_block[t_glob])
                    half = (b % 2) * D
                    pseg = pseg2[half:half + D, :]
                    co = (t_glob - base_j) * BLK
                    nc.tensor.matmul(
                        pseg, msg_j[:, tt, :], sm_sb_j[:, co:co + BLK],
                        start=bool(first_of_block[t_glob]),
                        stop=bool(last_of_block[t_glob]),
                    )
                    if last_of_block[t_glob]:
                        nc.vector.tensor_tensor(
                            out_sb[:, b * BLK:(b + 1) * BLK],
                            pseg,
                            atom_sb[:, b * BLK:(b + 1) * BLK],
                            mybir.AluOpType.add,
                        )

            xt_sb = None
            sm_sb = None
            chunk_base = -1
            n_p1 = (l2b * TPB) // p1_cols       # psum1 tiles per batch
            mm_p1 = p1_cols // 512              # 512-col MMs per psum1 tile
            # staircase chunking: ramp chunk sizes so the DMA lump never
            # outruns compute at the start
            ramp = [min(l2b, chunk_tiles), 16, 32, 48]
            chunk_starts = {}
            t = 0
            i = 0
            while t < ntiles:
                w = min(ramp[i] if i < len(ramp) else chunk_tiles, ntiles - t)
                chunk_starts[t] = w
                t += w
                i += 1
            out_frontier = 0
            for m2 in range(nb2):
                t0 = m2 * l2b
                if t0 in chunk_starts:
                    w = chunk_starts[t0]
                    xt_sb = xtp.tile([D, chunk_tiles * TPB], _BF16, tag="xt")
                    nc.sync.dma_start(
                        out=xt_sb[:, :w * TPB],
                        in_=xt_d[:, t0 * TPB: t0 * TPB + w * TPB],
                    )
                    sm_sb = smp.tile([TPB, chunk_tiles * BLK], _FP8, tag="sm")
                    nc.sync.dma_start(
                        out=sm_sb[:, :w * BLK],
                        in_=S_d[:, t0 * BLK: t0 * BLK + w * BLK],
                    )
                    chunk_base = t0
                if m2 == 1:
                    # atom features are first needed at the first block close;
                    # issuing here keeps the ring free for the first chunks
                    nc.sync.dma_start(out=atom_sb[:], in_=atomT_d[:, :])

                # ---- L1: feature-major, p1_cols-bond groups
                act1s = []
                for half in range(n_p1):
                    co = (t0 - chunk_base) * TPB + half * p1_cols
                    psum1 = psum1p.tile([2 * D, p1_cols], _F32, tag="psum1")
                    for q in range(mm_p1):
                        nc.tensor.matmul(
                            psum1[:, q * 512:(q + 1) * 512],
                            wg1_sb[0:D, :],
                            xt_sb[:, co + q * 512: co + (q + 1) * 512],
                            start=True, stop=True,
                        )
                    act1 = actp.tile([2 * D, p1_cols], _BF16, tag="act1")
                    nc.scalar.activation(
                        act1[:], psum1[:], AF.Silu, bias=b1g1_sb[:, 0:1], scale=1.0
                    )
                    act1s.append(act1)

                # ---- L2: bond-major, stationary = act1 slice
                if split_hg:
                    psum2h = psum2p.tile([TPB, l2b, D], _F32, tag="psum2h")
                    psum2g = psum2p.tile([TPB, l2b, D], _F32, tag="psum2g")
                    for tt in range(l2b):
                        a = act1s[(tt * TPB) // p1_cols]
                        sl = (tt * TPB) % p1_cols
                        for ps, wlo in ((psum2h, 0), (psum2g, D)):
                            nc.tensor.matmul(
                                ps[:, tt, :], a[:, sl:sl + TPB],
                                wg2_sb[:, wlo:wlo + D],
                                start=True, stop=not has_bias2,
                            )
                            if has_bias2:
                                nc.tensor.matmul(
                                    ps[:, tt, :], ones_sb[0:1, :],
                                    b2g2_sb[0:1, wlo:wlo + D],
                                    start=False, stop=True,
                                )
                else:
                    psum2 = psum2p.tile([TPB, l2b, 2 * D], _F32, tag="psum2")
                    psum2h = psum2[:, :, 0:D]
                    psum2g = psum2[:, :, D:2 * D]
                    for tt in range(l2b):
                        a = act1s[(tt * TPB) // p1_cols]
                        sl = (tt * TPB) % p1_cols
                        nc.tensor.matmul(
                            psum2[:, tt, :], a[:, sl:sl + TPB], wg2_sb[:, :],
                            start=True, stop=not has_bias2,
                        )
                        if has_bias2:
                            nc.tensor.matmul(
                                psum2[:, tt, :], ones_sb[0:1, :], b2g2_sb[0:1, :],
                                start=False, stop=True,
                            )

                # ---- activations: ACT silu on h half; gate hybrid:
                # most batches: 5-op stock-DVE smoothstep sigmoid
                # every ACT_GATE_EVERY-th batch: ACT tanh + DVE fixup
                # (balances ACT vs DVE busy; both LUT paths stay in the
                # silu_and_others table set)
                h2 = h2p.tile([TPB, l2b, D], _BF16, tag="h2")
                nc.scalar.activation(h2[:], psum2h[:, :, :] if split_hg else psum2h,
                                     AF.Silu)
                A = mybir.AluOpType
                msg = msgp.tile([TPB, l2b, D], _BF16, tag="msg")
                p2g = psum2g[:, :, :] if split_hg else psum2g
                sg = sgp.tile([TPB, l2b, D], _BF16, tag="sg")
                if gate_mode == "hybrid" and ((m2 * DVE_GATE_NUM) % DVE_GATE_DEN) < DVE_GATE_NUM:
                    # clamped odd cubic sigmoid, 7 DVE ops:
                    #   sg = clamp(0.5 + x(c1 + c2|x| + c3 x^2), 0, 1)
                    # via a1 = |x| (STT max(-x,x)); W/c3 = (a1 + c2/c3)*a1
                    c0 = sgp.tile([TPB, l2b, D], _BF16, tag="sgt1")
                    nc.vector.tensor_copy(c0[:], p2g)
                    a0 = sgp.tile([TPB, l2b, D], _BF16, tag="sgt2b")
                    nc.vector.tensor_scalar(a0[:], c0[:], -1.0, None, A.mult)
                    a1 = sgp.tile([TPB, l2b, D], _BF16, tag="sgt2")
                    nc.vector.tensor_tensor(a1[:], c0[:], a0[:], A.max)
                    a2 = sgp.tile([TPB, l2b, D], _BF16, tag="sgt3")
                    nc.vector.tensor_scalar(a2[:], a1[:], C2A / C3A, None, A.add)
                    a3 = sgp.tile([TPB, l2b, D], _BF16, tag="sgt4")
                    nc.vector.tensor_tensor(a3[:], a1[:], a2[:], A.mult)
                    a4 = sgp.tile([TPB, l2b, D], _BF16, tag="sgt5")
                    nc.vector.tensor_scalar(a4[:], a3[:], C3A, C1A, A.mult, A.add)
                    q_t = sgp.tile([TPB, l2b, D], _BF16, tag="sgt6")
                    nc.vector.tensor_tensor(q_t[:], c0[:], a4[:], A.mult)
                    cl = sgp.tile([TPB, l2b, D], _BF16, tag="sgt7")
                    nc.vector.tensor_scalar(cl[:], q_t[:], 0.5, 0.0, A.add, A.max)
                    nc.vector.tensor_scalar(sg[:], cl[:], 1.0, None, A.min)
                elif gate_mode == "dve":
                    sp_t = sgp.tile([TPB, l2b, D], _BF16, tag="sgt1")
                    nc.vector.tensor_scalar(sp_t[:], p2g, A_SS, 0.5, A.mult, A.add)
                    s_t = sgp.tile([TPB, l2b, D], _BF16, tag="sgt2")
                    nc.vector.tensor_scalar(s_t[:], sp_t[:], 0.0, 1.0, A.max, A.min)
                    u_t = sgp.tile([TPB, l2b, D], _BF16, tag="sgt3")
                    nc.vector.tensor_tensor(u_t[:], s_t[:], s_t[:], A.mult)
                    v_t = sgp.tile([TPB, l2b, D], _BF16, tag="sgt4")
                    nc.vector.tensor_scalar(v_t[:], s_t[:], -2.0, 3.0, A.mult, A.add)
                    nc.vector.tensor_tensor(sg[:], u_t[:], v_t[:], A.mult)
                else:
                    tg = sgp.tile([TPB, l2b, D], _BF16, tag="sgt1")
                    nc.scalar.activation(tg[:], p2g, AF.Tanh, scale=0.5)
                    nc.vector.tensor_scalar(sg[:], tg[:], 0.5, 0.5, A.mult, A.add)
                nc.gpsimd.tensor_tensor(msg[:], h2[:], sg[:], A.mult)
                stash[m2] = (msg, sm_sb, chunk_base)

                # ---- segment accumulation, skewed one batch for PE overlap
                if m2 > 0:
                    emit_seg(m2 - 1)
                    closed_upto = int(tile_block[m2 * l2b - 1])
                    if closed_upto - out_frontier >= 8:
                        nc.sync.dma_start(
                            out=out_d[:, out_frontier * BLK: closed_upto * BLK],
                            in_=out_sb[:, out_frontier * BLK: closed_upto * BLK],
                        )
                        out_frontier = closed_upto
            emit_seg(nb2 - 1)

            nc.sync.dma_start(
                out=out_d[:, out_frontier * BLK:],
                in_=out_sb[:, out_frontier * BLK:],
            )

    return nc


# ----------------------------------------------------------------- kernel()
LAST_EXEC_NS = None


def kernel(**inputs):
    atom_features = np.asarray(inputs["atom_features"], dtype=np.float32)
    bond_features = np.asarray(inputs["bond_features"], dtype=np.float32)
    bond_atom_indices = np.asarray(inputs["bond_atom_indices"])
    W1 = np.asarray(inputs["W1"], dtype=np.float32)
    W2 = np.asarray(inputs["W2"], dtype=np.float32)
    G1 = np.asarray(inputs["G1"], dtype=np.float32)
    G2 = np.asarray(inputs["G2"], dtype=np.float32)
    b1 = np.asarray(inputs["b1"], dtype=np.float32)
    b2 = np.asarray(inputs["b2"], dtype=np.float32)
    g1 = np.asarray(inputs["g1"], dtype=np.float32)
    g2 = np.asarray(inputs["g2"], dtype=np.float32)

    ntiles, tile_block, xt_list, S_list, atomT_list, assign = _plan_and_pack(
        atom_features, bond_features, bond_atom_indices
    )
    wg1, wg2, b1g1, b2g2 = _pack_weights(W1, G1, W2, G2, b1, g1, b2, g2)
    has_bias2 = not (np.all(b2 == 0.0) and np.all(g2 == 0.0))

    import os as _os
    gate_mode = _os.environ.get("KERNEL_GATE", "hybrid")
    nc = _build_nc(ntiles, tile_block, has_bias2, gate_mode,
                   l2b=16, p2_bufs=1, split_hg=True)

    in_maps = []
    for c in range(N_CORES):
        in_maps.append({
            "xt": xt_list[c],
            "smat": S_list[c],
            "atomT": atomT_list[c],
            "wg1": wg1,
            "wg2": wg2,
            "b1g1": b1g1,
            "b2g2": b2g2,
        })

    results, exec_ns = _run_timed(nc, in_maps, N_CORES)
    global LAST_EXEC_NS
    LAST_EXEC_NS = exec_ns

    out = np.empty((N_ATOMS, D), dtype=np.float32)
    blk_atoms = np.arange(BLK)
    for c in range(N_CORES):
        atom_idx = (assign[:, c][:, None] * BLK + blk_atoms[None, :]).ravel()
        valid = atom_idx < N_ATOMS
        out[atom_idx[valid]] = results[c]["out"][:, valid].T
    return out


def _run_timed(nc, in_maps, n_cores, reps=6):
    """Compile + run via PJRT on the axon neuron devices; inputs committed
    to devices up front, jitted NEFF executed `reps` extra times for wall
    timing (no NTFF hook in this axon build; donation off — the kernel
    writes every output element)."""
    import time as _time
    import sys as _sys
    import jax
    from jax.sharding import Mesh, PartitionSpec, NamedSharding
    from jax.experimental.shard_map import shard_map
    import concourse.mybir as _mybir
    from concourse import bass2jax as _b2j

    _b2j.install_neuronx_cc_hook()
    if nc.dbg_addr is not None:
        in_maps = [
            {**m, nc.dbg_addr.name: np.zeros((1, 2), np.uint32)} for m in in_maps
        ]

    partition_name = nc.partition_id_tensor.name if nc.partition_id_tensor else None

    in_names, out_names, out_avals, zero_outs = [], [], [], []
    for alloc in nc.m.functions[0].allocations:
        if not isinstance(alloc, _mybir.MemoryLocationSet):
            continue
        name = alloc.memorylocations[0].name
        if alloc.kind == "ExternalInput":
            if name != partition_name:
                in_names.append(name)
        elif alloc.kind == "ExternalOutput":
            shape = tuple(alloc.tensor_shape)
            dtype = _mybir.dt.np(alloc.dtype)
            out_names.append(name)
            out_avals.append(jax.core.ShapedArray(shape, dtype))
            zero_outs.append(np.zeros(shape, dtype))
    n_params = len(in_names)
    in_names_all = in_names + out_names
    if partition_name is not None:
        in_names_all.append(partition_name)

    def _body(*args):
        operands = list(args)
        if partition_name is not None:
            operands.append(_b2j.partition_id_tensor())
        outs = _b2j._bass_exec_p.bind(
            *operands,
            out_avals=tuple(out_avals),
            in_names=tuple(in_names_all),
            out_names=tuple(out_names),
            lowering_input_output_aliases=(),
            sim_require_finite=True,
            sim_require_nnan=True,
            nc=nc,
        )
        return tuple(outs)

    devices = jax.devices()[:n_cores]
    mesh = Mesh(np.asarray(devices), ("core",))
    n_outs = len(out_names)
    in_specs = (PartitionSpec("core"),) * (n_params + n_outs)
    out_specs = (PartitionSpec("core"),) * n_outs
    sharded = jax.jit(
        shard_map(_body, mesh=mesh, in_specs=in_specs, out_specs=out_specs,
                  check_rep=False),
        keep_unused=True,
    )

    sh = NamedSharding(mesh, PartitionSpec("core"))
    global_in = []
    for name in in_names:
        concat = np.concatenate([np.asarray(m[name]) for m in in_maps], axis=0)
        global_in.append(jax.device_put(concat, sh))
    for z in zero_outs:
        concat = np.zeros((n_cores * z.shape[0], *z.shape[1:]), z.dtype)
        global_in.append(jax.device_put(concat, sh))

    out_arrs = jax.block_until_ready(sharded(*global_in))  # compile + warm

    times = []
    for _ in range(reps):
        t0 = _time.perf_counter()
        o = sharded(*global_in)
        jax.block_until_ready(o)
        times.append(_time.perf_counter() - t0)
    print(f"[timing] reps(us): {[round(t*1e6,1) for t in times]}",
          file=_sys.stderr, flush=True)
    exec_ns = int(min(times) * 1e9)

    results = []
    for c in range(n_cores):
        results.append({
            name: np.asarray(out_arrs[i]).reshape(n_cores, *out_avals[i].shape)[c]
            for i, name in enumerate(out_names)
        })
    return results, exec_ns
